# revision 21
# baseline (speedup 1.0000x reference)
"""GCN layer (GCNConv + bias + PReLU) on 8 Trainium2 NeuronCores.

Algorithm
---------
Reference:  out = PReLU( segsum_dst( dis[src]*dis[dst] * (X W)[src] ) + b )
with dis = 1/sqrt(in-degree) (0 where deg==0).

Factorization used here:
  xs   = (X @ W) * dis[:, None]                  (per-node scaling, launch 1)
  agg[d] = sum_{e: dst(e)=d} xs[src(e)]          (pure segment sum, launch 2)
  out[d] = PReLU( dis[d]*agg[d] + b )

Distribution: launch 1 shards nodes by contiguous range (each core computes
xs for its 12.5k rows); the host assembles the full xs and replicates it.
Launch 2 shards destination nodes across cores by degree-striped ranks
(rank i -> core i%8) so every core gets an identical degree mixture.

Per core, launch 2 uses an ELL/SELL-style grid: dst nodes sorted by degree
(desc) are packed into blocks of 128 (partition dim) x groups of 8 blocks.
For each group, grid row r holds the r-th incoming edge's source id for
each of the 1024 dst slots (dummy -> index N, a zero row in xs).  The rows
are fetched with one indirect DMA (gather) per 8 rows, and reduced into
PSUM with identity matmuls: psum[slot, feat] += I.T @ msg_row.  The output
is written densely; the host undoes the permutation.
"""

import os

import numpy as np

from concourse import bacc, bass, mybir, tile
from concourse.bass import IndirectOffsetOnAxis
from concourse.bass_utils import run_bass_kernel_spmd

# Filled in by kernel() on every call: wall/HW times for inspection by tests.
LAST_RUN_INFO = {}

F32 = mybir.dt.float32
I32 = mybir.dt.int32

N_NODES = 100000
N_EDGES = 3200000
IN_DIM = 512
OUT_DIM = 64
NCORES = 8
K4 = IN_DIM // 128  # k-chunks for the feature matmul

M_C = N_NODES // NCORES           # dst nodes per core (12500)
NBLK = (M_C + 127) // 128         # 128-wide dst blocks per core (98)
M_PAD = NBLK * 128                # padded dst slots per core (12544)
BPG = 8                           # blocks per group (psum free dim 512)
CHUNK_ROWS = 8                    # grid rows fetched per gather DMA
RR = 32767                        # real nodes per int16-addressable range
NR = (N_NODES + RR - 1) // RR     # source ranges (4)
XS_ROWS = 32768 * NR              # padded xs buffer (zero row per range)
I16 = mybir.dt.int16

# launch-1 row-chunk quarters (chunks of 128 rows, grouped for big DMAs)
def _quarters(nchunks, q=4):
    base = nchunks // q
    sizes = [base] * q
    for i in range(nchunks - base * q):
        sizes[i] += 1
    out, c0 = [], 0
    for s in sizes:
        if s:
            out.append((c0, s))
        c0 += s
    return out


# ----------------------------------------------------------------------------
# launch 1: xs = (X @ W) * dis   (per-core row slab)
# ----------------------------------------------------------------------------
def build_launch1():
    nc = bacc.Bacc("TRN2", debug=False, enable_asserts=False,
                   num_devices=NCORES)
    fT = nc.dram_tensor("fT", [IN_DIM, M_PAD], F32, kind="ExternalInput")
    Wt = nc.dram_tensor("W", [IN_DIM, OUT_DIM], F32, kind="ExternalInput")
    dis = nc.dram_tensor("dis", [128, NBLK], F32, kind="ExternalInput")
    xs = nc.dram_tensor("xs", [128, NBLK * OUT_DIM], F32,
                        kind="ExternalOutput")

    with tile.TileContext(nc) as tc:
        with (
            tc.tile_pool(name="const", bufs=1) as cpool,
            tc.tile_pool(name="feat", bufs=2) as fpool,
            tc.tile_pool(name="stage", bufs=1) as spool,
            tc.tile_pool(name="psum", bufs=4, space="PSUM") as ppool,
        ):
            wt = cpool.tile([128, K4, OUT_DIM], F32)
            nc.sync.dma_start(
                out=wt[:],
                in_=Wt.ap().rearrange("(k p) o -> p k o", p=128))
            dt_ = cpool.tile([128, NBLK], F32)
            nc.sync.dma_start(out=dt_[:], in_=dis.ap()[:, :])
            stage = spool.tile([128, NBLK * OUT_DIM], F32)


            # fT rows are (k4*128 + p); load a whole quarter with ONE DMA so
            # downstream matmuls carry few semaphore waits (walrus limit).
            fT_v = fT.ap().rearrange("(k p) m -> p k m", p=128)
            for c0, nch in _quarters(NBLK):
                ft = fpool.tile([128, K4, nch * 128], F32, tag="f")
                nc.sync.dma_start(
                    out=ft[:],
                    in_=fT_v[:, :, c0 * 128:(c0 + nch) * 128])
                for ch in range(nch):
                    ps = ppool.tile([128, OUT_DIM], F32)
                    for k in range(K4):
                        nc.tensor.matmul(
                            out=ps[:],
                            lhsT=ft[:, k, ch * 128:(ch + 1) * 128],
                            rhs=wt[:, k, :],
                            start=(k == 0), stop=(k == K4 - 1))
                    cg = c0 + ch
                    nc.vector.tensor_scalar_mul(
                        out=stage[:, cg * OUT_DIM:(cg + 1) * OUT_DIM],
                        in0=ps[:], scalar1=dt_[:, cg:cg + 1])
            nc.sync.dma_start(out=xs.ap()[:, :], in_=stage[:])
    nc.finalize()
    return nc


# ----------------------------------------------------------------------------
# launch 2: gather + segment-sum + epilogue
# ----------------------------------------------------------------------------
def build_launch2(group_shapes, tot16):
    """group_shapes: list of (nb, [R_q for q in range(NR)]) per group.
    tot16 = total int16-wrapped index columns."""
    nc = bacc.Bacc("TRN2", debug=False, enable_asserts=False,
                   num_devices=NCORES)
    xs = nc.dram_tensor("xs", [XS_ROWS, OUT_DIM], F32, kind="ExternalInput")
    idx = nc.dram_tensor("idx", [128, tot16], I16, kind="ExternalInput")
    disg = nc.dram_tensor("disg", [128, NBLK], F32, kind="ExternalInput")
    b8 = nc.dram_tensor("b8", [128, BPG * OUT_DIM], F32,
                        kind="ExternalInput")
    arep = nc.dram_tensor("arep", [128, 1], F32, kind="ExternalInput")
    ident = nc.dram_tensor("ident", [128, 128], F32, kind="ExternalInput")
    out = nc.dram_tensor("out", [128, NBLK * OUT_DIM], F32,
                         kind="ExternalOutput")

    with tile.TileContext(nc) as tc:
        with (
            tc.tile_pool(name="const", bufs=1) as cpool,
            tc.tile_pool(name="ixp", bufs=4) as ixpool,
            tc.tile_pool(name="msg", bufs=3) as mpool,
            tc.tile_pool(name="tmp", bufs=2) as tpool,
            tc.tile_pool(name="stage", bufs=1) as spool,
            tc.tile_pool(name="psum", bufs=2, space="PSUM") as ppool,
        ):
            it = cpool.tile([128, 128], F32)
            nc.sync.dma_start(out=it[:], in_=ident.ap()[:, :])
            dg = cpool.tile([128, NBLK], F32)
            nc.sync.dma_start(out=dg[:], in_=disg.ap()[:, :])
            bt = cpool.tile([128, BPG * OUT_DIM], F32)
            nc.sync.dma_start(out=bt[:], in_=b8.ap()[:, :])
            at = cpool.tile([128, 1], F32)
            nc.sync.dma_start(out=at[:], in_=arep.ap()[:, :])
            stage = spool.tile([128, NBLK * OUT_DIM], F32)

            xsv = [xs.ap()[32768 * q:32768 * (q + 1), :] for q in range(NR)]
            off16 = 0     # column offset into idx (int16-wrapped units)
            gb0 = 0       # global block index
            for nb, Rqs in group_shapes:
                nfree = nb * OUT_DIM
                ps = ppool.tile([128, BPG * OUT_DIM], F32, tag="psum")
                chunks = []
                for q in range(NR):
                    r = 0
                    while r < Rqs[q]:
                        rk = min(CHUNK_ROWS, Rqs[q] - r)
                        chunks.append((q, rk))
                        r += rk
                for ci, (q, rk) in enumerate(chunks):
                    n_idx = rk * nb * 128
                    w16 = n_idx // 16
                    ixt = ixpool.tile([128, CHUNK_ROWS * BPG * 8], I16,
                                      tag="ix")
                    nc.sync.dma_start(out=ixt[:, :w16],
                                      in_=idx.ap()[:, off16:off16 + w16])
                    msg = mpool.tile([128, CHUNK_ROWS * BPG, OUT_DIM], F32,
                                     tag="msg")
                    nc.gpsimd.dma_gather(
                        out_ap=msg[:, :rk * nb, :],
                        in_ap=xsv[q],
                        idxs_ap=ixt[:, :w16],
                        num_idxs=n_idx,
                        num_idxs_reg=n_idx,
                        elem_size=OUT_DIM,
                        single_packet=False)
                    for rl in range(rk):
                        nc.tensor.matmul(
                            out=ps[:, :nfree],
                            lhsT=it[:],
                            rhs=msg[:, rl * nb:(rl + 1) * nb, :],
                            start=(ci == 0 and rl == 0),
                            stop=(ci == len(chunks) - 1 and rl == rk - 1))
                    off16 += w16
                # epilogue: t0 = psum * dis ; t0 += b ; prelu -> stage
                t0 = tpool.tile([128, BPG * OUT_DIM], F32, tag="t0")
                for bi in range(nb):
                    nc.vector.tensor_scalar_mul(
                        out=t0[:, bi * OUT_DIM:(bi + 1) * OUT_DIM],
                        in0=ps[:, bi * OUT_DIM:(bi + 1) * OUT_DIM],
                        scalar1=dg[:, gb0 + bi:gb0 + bi + 1])
                nc.vector.tensor_tensor(
                    out=t0[:, :nfree], in0=t0[:, :nfree],
                    in1=bt[:, :nfree], op=mybir.AluOpType.add)
                tn = tpool.tile([128, BPG * OUT_DIM], F32, tag="tn")
                nc.vector.tensor_scalar(
                    out=tn[:, :nfree], in0=t0[:, :nfree],
                    scalar1=0.0, scalar2=at[:, 0:1],
                    op0=mybir.AluOpType.min, op1=mybir.AluOpType.mult)
                nc.vector.tensor_scalar_max(
                    out=t0[:, :nfree], in0=t0[:, :nfree], scalar1=0.0)
                nc.vector.tensor_tensor(
                    out=stage[:, gb0 * OUT_DIM:gb0 * OUT_DIM + nfree],
                    in0=t0[:, :nfree], in1=tn[:, :nfree],
                    op=mybir.AluOpType.add)
                gb0 += nb
            nc.sync.dma_start(out=out.ap()[:, :], in_=stage[:])
    nc.finalize()
    return nc


# ----------------------------------------------------------------------------
# host-side graph preprocessing
# ----------------------------------------------------------------------------
def _wrap16(flat):
    """dma_gather index layout: flat index i lives at [i % 16, i // 16],
    replicated to all 8 Q7 cores (partitions 16k..16k+15)."""
    n = flat.shape[0]
    arr = flat.reshape(n // 16, 16).T.astype(np.int16)   # [16, n//16]
    return np.tile(arr, (8, 1))                          # [128, n//16]


def preprocess(edge_index):
    row = np.asarray(edge_index[0], dtype=np.int64)
    col = np.asarray(edge_index[1], dtype=np.int64)
    deg = np.bincount(col, minlength=N_NODES).astype(np.int64)
    dis = np.where(deg > 0, 1.0 / np.sqrt(np.maximum(deg, 1)), 0.0)
    dis = dis.astype(np.float32)

    # CSR by (destination, then source) so per-dst ranges split cleanly by
    # source range q = src // RR
    q_of = row // RR
    degq = np.bincount(col * NR + q_of,
                       minlength=N_NODES * NR).reshape(N_NODES, NR)
    e_ord = np.lexsort((row, col))
    srcs = np.ascontiguousarray(row[e_ord])
    starts = np.zeros(N_NODES + 1, dtype=np.int64)
    np.cumsum(deg, out=starts[1:])
    cumq = np.zeros((N_NODES, NR + 1), dtype=np.int64)
    np.cumsum(degq, axis=1, out=cumq[:, 1:])

    order = np.argsort(-deg, kind="stable")  # degree-descending node ranks

    # per-core dst nodes in degree order, padded to M_PAD with -1
    nodes = np.full((NCORES, M_PAD), -1, dtype=np.int64)
    for c in range(NCORES):
        nodes[c, :M_C] = order[c::NCORES]
    nclip = np.maximum(nodes, 0)
    ndegq = np.where(nodes[:, :, None] >= 0, degq[nclip], 0)  # [C, M_PAD, NR]

    group_blocks = []
    b0 = 0
    while b0 < NBLK:
        group_blocks.append((b0, min(BPG, NBLK - b0)))
        b0 += BPG
    group_shapes = []
    for b0, nb in group_blocks:
        sl = ndegq[:, b0 * 128:(b0 + nb) * 128, :]
        Rqs = [int(sl[:, :, q].max()) for q in range(NR)]
        if sum(Rqs) == 0:
            Rqs[0] = 1  # keep >=1 row so the psum group gets initialized
        group_shapes.append((nb, Rqs))
    tot16 = sum(sum(R * nb * 8 for R in Rqs) for nb, Rqs in group_shapes)

    idx_all = np.empty((NCORES, 128, tot16), dtype=np.int16)
    disg_all = np.zeros((NCORES, 128, NBLK), dtype=np.float32)
    DUMMY = np.int16(RR)  # per-range zero row (local id 32767)
    for c in range(NCORES):
        off = 0
        for (b0, nb), (_, Rqs) in zip(group_blocks, group_shapes):
            ng = nodes[c, b0 * 128:(b0 + nb) * 128].reshape(nb, 128)
            ngc = np.maximum(ng, 0)
            for q in range(NR):
                R = Rqs[q]
                if R == 0:
                    continue
                dgg = np.where(ng >= 0, degq[ngc, q], 0)       # [nb, 128]
                st = np.where(ng >= 0, starts[ngc] + cumq[ngc, q], 0)
                r = np.arange(R, dtype=np.int64)[:, None, None]
                pos = np.minimum(st[None] + r, N_EDGES - 1)
                grid = np.where(r < dgg[None], srcs[pos] % RR, DUMMY)
                w16 = R * nb * 8
                idx_all[c, :, off:off + w16] = _wrap16(
                    grid.reshape(-1).astype(np.int16))
                off += w16
            disg_all[c, :, b0:b0 + nb] = np.where(
                ng >= 0, dis[ngc], 0.0).T
        assert off == tot16, (off, tot16)
    return dis, order, nodes, group_shapes, tot16, idx_all, disg_all


_L1_CACHE = {}
_L2_CACHE = {}


def kernel(features, edge_index, W, b, prelu_a):
    features = np.asarray(features, dtype=np.float32)
    edge_index = np.asarray(edge_index)
    W = np.asarray(W, dtype=np.float32)
    b = np.asarray(b, dtype=np.float32)
    prelu_a = np.asarray(prelu_a, dtype=np.float32)

    (dis, order, nodes, group_shapes, tot16, idx_all,
     disg_all) = preprocess(edge_index)

    # ---------------- launch 1: xs slabs ----------------
    if "nc" not in _L1_CACHE:
        _L1_CACHE["nc"] = build_launch1()
    nc1 = _L1_CACHE["nc"]

    fT = np.ascontiguousarray(features.T)  # [512, N]
    in_maps1 = []
    for c in range(NCORES):
        slab = np.zeros((IN_DIM, M_PAD), dtype=np.float32)
        slab[:, :M_C] = fT[:, c * M_C:(c + 1) * M_C]
        dslab = np.zeros((128, NBLK), dtype=np.float32)
        dv = dis[c * M_C:(c + 1) * M_C]
        dslab.reshape(-1)[:0] = 0  # noop, keep layout explicit below
        dpad = np.zeros(M_PAD, dtype=np.float32)
        dpad[:M_C] = dv
        dslab[:] = dpad.reshape(NBLK, 128).T
        in_maps1.append({"fT": slab, "W": W, "dis": dslab})
    trace = os.environ.get("GCN_TRACE") == "1"
    import time as _time
    _t0 = _time.monotonic()
    res1 = run_bass_kernel_spmd(nc1, in_maps1, core_ids=list(range(NCORES)),
                                trace=trace)
    LAST_RUN_INFO["launch1_wall_s"] = _time.monotonic() - _t0
    LAST_RUN_INFO["launch1_exec_ns"] = res1.exec_time_ns

    xs_rows = np.empty((N_NODES, OUT_DIM), dtype=np.float32)
    for c in range(NCORES):
        slab = res1.results[c]["xs"].reshape(128, NBLK, OUT_DIM)
        rows = slab.transpose(1, 0, 2).reshape(M_PAD, OUT_DIM)[:M_C]
        xs_rows[c * M_C:(c + 1) * M_C] = rows
    xs_buf = np.zeros((XS_ROWS, OUT_DIM), dtype=np.float32)
    all_n = np.arange(N_NODES, dtype=np.int64)
    xs_buf[(all_n // RR) * 32768 + all_n % RR] = xs_rows

    # ---------------- launch 2: segment sum + epilogue ----------------
    key = (tuple((nb, tuple(R)) for nb, R in group_shapes), tot16)
    if _L2_CACHE.get("key") != key:
        _L2_CACHE["nc"] = build_launch2(group_shapes, tot16)
        _L2_CACHE["key"] = key
    nc2 = _L2_CACHE["nc"]

    b8 = np.tile(b, (128, BPG)).astype(np.float32)
    arep = np.full((128, 1), float(prelu_a.reshape(-1)[0]), dtype=np.float32)
    ident = np.eye(128, dtype=np.float32)
    in_maps2 = []
    for c in range(NCORES):
        in_maps2.append({
            "xs": xs_buf, "idx": idx_all[c], "disg": disg_all[c],
            "b8": b8, "arep": arep, "ident": ident,
        })
    _t0 = _time.monotonic()
    res2 = run_bass_kernel_spmd(nc2, in_maps2, core_ids=list(range(NCORES)),
                                trace=trace)
    LAST_RUN_INFO["launch2_wall_s"] = _time.monotonic() - _t0
    LAST_RUN_INFO["launch2_exec_ns"] = res2.exec_time_ns

    out_full = np.zeros((N_NODES, OUT_DIM), dtype=np.float32)
    for c in range(NCORES):
        slab = res2.results[c]["out"].reshape(128, NBLK, OUT_DIM)
        rows = slab.transpose(1, 0, 2).reshape(M_PAD, OUT_DIM)[:M_C]
        out_full[nodes[c, :M_C]] = rows
    return out_full
